# revision 9
# baseline (speedup 1.0000x reference)
"""Trainium2 Bass kernel for sliding-window GQA attention (nn_Attention_9861244911852).

Sharding: 8 cores = 2 batches x 4 sequence chunks of 512 rows.
Each core: q/k/v projections (own 512 rows, all heads, bf16 matmuls, f32 psum),
fused rmsnorm+rope via host tables, kv AllGather across the 4-core batch group,
full-T masked attention (host-computed additive masks keep the graph core-uniform),
output projection. Host concatenates the disjoint [512, 2560] output chunks.
"""
import sys

sys.path.insert(0, '/opt/trn_rl_repo')

import numpy as np
import ml_dtypes

B, T, D, N, KH, H = 2, 2048, 2560, 8, 4, 256
WINDOW = 1024
ROPE_BASE = 10000.0
CHUNK = 512            # query rows per core
NCORES = 8
GROUPS = [[0, 1, 2, 3], [4, 5, 6, 7]]
NEG = np.float32(-1.0e38)
NT = CHUNK // 128      # 4 q-tiles per core
ST = T // 128          # 16 s-tiles (full batch)
DC = D // 128          # 20 contraction chunks
DP = H // 2            # 128 rope pairs
BF = ml_dtypes.bfloat16

_BUILD_CACHE = {}


# ----------------------------------------------------------------- host prep
def _perm():
    return np.concatenate([np.arange(0, H, 2), np.arange(1, H, 2)])


def _prep_shared(w_q, w_kv, q_scale, k_scale, w_o):
    p = _perm()
    wq = np.ascontiguousarray(w_q[:, :, p].transpose(1, 0, 2).reshape(D, N * H).astype(BF))
    wk = w_kv[0][:, :, p].transpose(1, 0, 2).reshape(D, KH * H).astype(BF)
    wv = w_kv[1].transpose(1, 0, 2).reshape(D, KH * H).astype(BF)
    wkv = np.ascontiguousarray(np.concatenate([wk, wv], axis=1))   # [D, 2048]
    wo = np.ascontiguousarray(w_o.reshape(N * H, D).astype(BF))
    qs = q_scale[p].astype(np.float32)
    ks = k_scale[p].astype(np.float32)
    return wq, wkv, wo, qs, ks


def _rope_tabs(pos, qs, ks):
    """pos [CHUNK] int32 -> [8, 128, CHUNK] f32 tables (qA qB qC qD kA kB kC kD).

    Permuted-head layout: row i of the top half holds component 2i, bottom 2i+1.
    q tables fold in q_scale and H**-0.5; k tables fold k_scale.
    """
    inv = (1.0 / ROPE_BASE ** (np.arange(DP, dtype=np.float32) / np.float32(DP))).astype(np.float32)
    ang = pos.astype(np.float32)[None, :] * inv[:, None]          # [128, CHUNK]
    c = np.cos(ang).astype(np.float32)
    s = np.sin(ang).astype(np.float32)
    s16 = np.float32(H ** -0.5)
    qt, qb = qs[:DP, None], qs[DP:, None]
    kt, kb = ks[:DP, None], ks[DP:, None]
    return np.stack([c * qt * s16, -s * qb * s16, s * qt * s16, c * qb * s16,
                     c * kt, -s * kb, s * kt, c * kb]).astype(np.float32)


def _apen(j):
    """Additive attention mask for seq-chunk j: [NT, 128, T] f32 (0 or NEG)."""
    t = (CHUNK * j + np.arange(CHUNK)).reshape(NT, 128)
    s = np.arange(T)
    d = t[:, :, None] - s[None, None, :]
    valid = (d >= 0) & (d < WINDOW)
    return np.where(valid, np.float32(0.0), NEG).astype(np.float32)


def _prep_in_maps(x, positions, w_q, w_kv, q_scale, k_scale, w_o):
    wq, wkv, wo, qs, ks = _prep_shared(w_q, w_kv, q_scale, k_scale, w_o)
    ident = np.eye(128, dtype=np.float32).astype(BF)
    in_maps = []
    for c in range(NCORES):
        b, j = c // 4, c % 4
        rows = slice(CHUNK * j, CHUNK * (j + 1))
        xt = np.ascontiguousarray(x[b, rows, :].T.astype(BF))
        rope = _rope_tabs(np.asarray(positions)[b, rows], qs, ks)
        in_maps.append({
            "xt": xt, "wq": wq, "wkv": wkv, "wo": wo,
            "rope": rope, "apen": _apen(j),
            "ident": ident,
            "onesc": np.ones((128, 1), np.float32),
            "onesr": np.ones((1, 128), np.float32),
        })
    return in_maps


# --------------------------------------------------------------- bass kernel
def _build():
    import concourse.bass as bass
    import concourse.mybir as mybir
    from concourse import bacc, tile

    f32 = mybir.dt.float32
    bf16 = mybir.dt.bfloat16

    nc = bacc.Bacc(None, target_bir_lowering=False)

    xt_e = nc.declare_dram_parameter("xt", [D, CHUNK], bf16, isOutput=False)
    wq_e = nc.declare_dram_parameter("wq", [D, N * H], bf16, isOutput=False)
    wkv_e = nc.declare_dram_parameter("wkv", [D, 2 * KH * H], bf16, isOutput=False)
    wo_e = nc.declare_dram_parameter("wo", [N * H, D], bf16, isOutput=False)
    rope_e = nc.declare_dram_parameter("rope", [8, 128, CHUNK], f32, isOutput=False)
    apen_e = nc.declare_dram_parameter("apen", [NT, 128, T], f32, isOutput=False)
    ident_e = nc.declare_dram_parameter("ident", [128, 128], bf16, isOutput=False)
    onesc_e = nc.declare_dram_parameter("onesc", [128, 1], f32, isOutput=False)
    onesr_e = nc.declare_dram_parameter("onesr", [1, 128], f32, isOutput=False)
    out_e = nc.declare_dram_parameter("out", [CHUNK, D], f32, isOutput=True)

    k_local = nc.dram_tensor("k_local", [KH * H, CHUNK], bf16)   # [1024 h, 512 s]
    v_local = nc.dram_tensor("v_local", [CHUNK, KH * H], bf16)   # [512 s, 1024 h]
    k_ag = nc.dram_tensor("k_ag", [4 * KH * H, CHUNK], bf16)     # rank-major
    v_ag = nc.dram_tensor("v_ag", [4 * CHUNK, KH * H], bf16)

    Exp = mybir.ActivationFunctionType.Exp
    Sqrt = mybir.ActivationFunctionType.Sqrt
    mult = mybir.AluOpType.mult
    add_op = mybir.AluOpType.add

    with tile.TileContext(nc) as tc:
      with tc.tile_pool(name="const", bufs=1) as constp, \
           tc.tile_pool(name="qt", bufs=1) as qtp, \
           tc.tile_pool(name="enc_sb", bufs=1) as encsbp:

        onesc = constp.tile([128, 1], f32, tag="onesc")
        nc.sync.dma_start(onesc[:], onesc_e[:])
        eps_t = constp.tile([1, 1], f32, tag="eps")
        nc.gpsimd.memset(eps_t[:], 1e-6)
        onesr = constp.tile([1, 128], f32, tag="onesr")
        nc.sync.dma_start(onesr[:], onesr_e[:])
        ident = constp.tile([128, 128], bf16, tag="ident")
        nc.sync.dma_start(ident[:], ident_e[:])

        qT = {}    # (qh, hc) -> bf16 [128, CHUNK]
        encT = {}  # (qh, half) -> bf16 [128, CHUNK]

        # ================= projections + kv AllGathers =================
        with tc.tile_pool(name="xtp", bufs=1) as xtp, \
             tc.tile_pool(name="wslab", bufs=1) as wsp, \
             tc.tile_pool(name="ropep", bufs=1) as ropep, \
             tc.tile_pool(name="qkv_ps", bufs=4, space="PSUM") as qkvp, \
             tc.tile_pool(name="ss_ps", bufs=2, space="PSUM") as ssp, \
             tc.tile_pool(name="aux_ps", bufs=2, space="PSUM") as auxp, \
             tc.tile_pool(name="scr", bufs=4) as scrp, \
             tc.tile_pool(name="kvl", bufs=1) as kvlp:

            rope_sb = []
            for ri in range(8):
                rt = ropep.tile([128, CHUNK], f32, tag=f"rope{ri}", name=f"rope{ri}")
                nc.sync.dma_start(rt[:], rope_e[ri])
                rope_sb.append(rt)

            xt_sb = []
            for dc in range(DC):
                xtile = xtp.tile([128, CHUNK], bf16, tag=f"xt{dc}", name=f"xt{dc}")
                nc.sync.dma_start(xtile[:], xt_e[dc * 128:(dc + 1) * 128, :])
                xt_sb.append(xtile)

            # weight slabs: [128, 2048] rows (4KB descriptors), resident per phase
            wkv_sb = []
            for dc in range(DC):
                wt = wsp.tile([128, 2048], bf16, tag=f"wkv{dc}", name=f"wkv{dc}")
                nc.sync.dma_start(wt[:], wkv_e[dc * 128:(dc + 1) * 128, :])
                wkv_sb.append(wt)

            def proj_unit(slabs, col0):
                """psum [128, CHUNK] = w[:, col0:col0+128].T @ xT (20 accum matmuls)."""
                ps = qkvp.tile([128, CHUNK], f32, tag="qkv", name="qkvps")
                for dc in range(DC):
                    nc.tensor.matmul(ps[:], slabs[dc][:, col0:col0 + 128], xt_sb[dc][:],
                                     start=(dc == 0), stop=(dc == DC - 1))
                return ps

            def norm_rope(p_top, p_bot, tabs, out_top, out_bot):
                """rmsnorm (f32) + rope tables + cast bf16."""
                sq_t = scrp.tile([128, CHUNK], f32, tag="sq", name="sqt")
                nc.scalar.square(sq_t[:], p_top[:])
                ss = ssp.tile([1, CHUNK], f32, tag="ss", name="ss")
                nc.tensor.matmul(ss[:], onesc[:], sq_t[:], start=True, stop=False)
                sq_b = scrp.tile([128, CHUNK], f32, tag="sq", name="sqb")
                nc.scalar.square(sq_b[:], p_bot[:])
                nc.tensor.matmul(ss[:], onesc[:], sq_b[:], start=False, stop=True)
                std = scrp.tile([1, CHUNK], f32, tag="std", name="std")
                nc.scalar.activation(std[:], ss[:], Sqrt, bias=eps_t[:], scale=1.0 / H)
                rsb = scrp.tile([1, CHUNK], f32, tag="rsb", name="rsb")
                nc.vector.reciprocal(rsb[:], std[:])
                rb = auxp.tile([128, CHUNK], f32, tag="aux", name="rb")
                nc.tensor.matmul(rb[:], onesr[:], rsb[:], start=True, stop=True)
                A, Bt, C, Dt = tabs
                t1 = scrp.tile([128, CHUNK], f32, tag="t1", name="t1")
                t2 = scrp.tile([128, CHUNK], f32, tag="t2", name="t2")
                nc.vector.tensor_tensor(t1[:], p_top[:], A[:], mult)
                nc.vector.tensor_tensor(t2[:], p_bot[:], Bt[:], mult)
                nc.vector.tensor_tensor(t1[:], t1[:], t2[:], add_op)
                nc.vector.tensor_tensor(out_top[:], t1[:], rb[:], mult)
                t3 = scrp.tile([128, CHUNK], f32, tag="t1", name="t3")
                t4 = scrp.tile([128, CHUNK], f32, tag="t2", name="t4")
                nc.vector.tensor_tensor(t3[:], p_top[:], C[:], mult)
                nc.vector.tensor_tensor(t4[:], p_bot[:], Dt[:], mult)
                nc.vector.tensor_tensor(t3[:], t3[:], t4[:], add_op)
                nc.vector.tensor_tensor(out_bot[:], t3[:], rb[:], mult)

            # ---- k projection + epilogue -> k_local -> AllGather(k)
            for kh in range(KH):
                p_top = proj_unit(wkv_sb, kh * H)
                p_bot = proj_unit(wkv_sb, kh * H + 128)
                k_top = kvlp.tile([128, CHUNK], bf16, tag="ktop")
                k_bot = kvlp.tile([128, CHUNK], bf16, tag="kbot")
                norm_rope(p_top, p_bot, rope_sb[4:8], k_top, k_bot)
                nc.sync.dma_start(k_local[kh * H:kh * H + 128, :], k_top[:])
                nc.sync.dma_start(k_local[kh * H + 128:kh * H + 256, :], k_bot[:])

            nc.gpsimd.collective_compute(
                "AllGather", mybir.AluOpType.bypass, replica_groups=GROUPS,
                ins=[k_local[:]], outs=[k_ag[:]])

            # ---- v projection -> transpose -> v_local -> AllGather(v)
            for kh in range(KH):
                for hc in range(2):
                    ps = proj_unit(wkv_sb, 1024 + kh * H + hc * 128)
                    v_sb = scrp.tile([128, CHUNK], bf16, tag="vsb", name="vsb")
                    nc.scalar.copy(v_sb[:], ps[:])
                    vt_ps = auxp.tile([128, CHUNK], bf16, tag="aux", name="vtps")
                    for stl in range(4):
                        nc.tensor.transpose(vt_ps[:, stl * 128:(stl + 1) * 128],
                                            v_sb[:, stl * 128:(stl + 1) * 128], ident[:])
                    vt_loc = scrp.tile([128, CHUNK], bf16, tag="vtsb", name="vtloc")
                    nc.scalar.copy(vt_loc[:], vt_ps[:])
                    for stl in range(4):
                        nc.sync.dma_start(
                            v_local[stl * 128:(stl + 1) * 128,
                                    kh * H + hc * 128:kh * H + (hc + 1) * 128],
                            vt_loc[:, stl * 128:(stl + 1) * 128])

            nc.gpsimd.collective_compute(
                "AllGather", mybir.AluOpType.bypass, replica_groups=GROUPS,
                ins=[v_local[:]], outs=[v_ag[:]])

            # ---- q projection (overlaps the AllGathers)
            wq_sb = []
            for dc in range(DC):
                wt = wsp.tile([128, 2048], bf16, tag=f"wkv{dc}", name=f"wq{dc}")
                nc.sync.dma_start(wt[:], wq_e[dc * 128:(dc + 1) * 128, :])
                wq_sb.append(wt)
            for qh in range(N):
                p_top = proj_unit(wq_sb, qh * H)
                p_bot = proj_unit(wq_sb, qh * H + 128)
                q_top = qtp.tile([128, CHUNK], bf16, tag=f"q{qh}t", name=f"q{qh}t")
                q_bot = qtp.tile([128, CHUNK], bf16, tag=f"q{qh}b", name=f"q{qh}b")
                norm_rope(p_top, p_bot, rope_sb[0:4], q_top, q_bot)
                qT[(qh, 0)] = q_top
                qT[(qh, 1)] = q_bot

        # ================= attention =================
        with tc.tile_pool(name="apen_p", bufs=1) as apenp, \
             tc.tile_pool(name="kvw", bufs=1) as kvwp, \
             tc.tile_pool(name="psoft", bufs=2) as psp, \
             tc.tile_pool(name="ptb", bufs=2) as ptp, \
             tc.tile_pool(name="lg_ps", bufs=2, space="PSUM") as lgp, \
             tc.tile_pool(name="tr_ps", bufs=2, space="PSUM") as trp, \
             tc.tile_pool(name="enc_ps", bufs=2, space="PSUM") as encp, \
             tc.tile_pool(name="dstat", bufs=4) as dsp:

            apen_sb = []
            for ti in range(NT):
                at = apenp.tile([128, T], f32, tag=f"ap{ti}", name=f"ap{ti}")
                nc.sync.dma_start(at[:], apen_e[ti])
                apen_sb.append(at)

            # kT: 8 tiles [128, T] over (kh, hc); vT: 16 tiles [128, 1024]
            kt_sb = {}
            for kh in range(KH):
                for hc in range(2):
                    kt = kvwp.tile([128, T], bf16, tag=f"kt{kh}_{hc}", name=f"kt{kh}_{hc}")
                    for r in range(4):
                        nc.sync.dma_start(
                            kt[:, r * CHUNK:(r + 1) * CHUNK],
                            k_ag[r * 1024 + kh * H + hc * 128:
                                 r * 1024 + kh * H + (hc + 1) * 128, :])
                    kt_sb[(kh, hc)] = kt
            vt_sb = []
            for st in range(ST):
                r, stl = st // 4, st % 4
                vt = kvwp.tile([128, KH * H], bf16, tag=f"vt{st}", name=f"vt{st}")
                nc.sync.dma_start(
                    vt[:], v_ag[r * CHUNK + stl * 128:r * CHUNK + (stl + 1) * 128, :])
                vt_sb.append(vt)

            for kh in range(KH):
                for qh in (2 * kh, 2 * kh + 1):
                    # pt layout: [128 s-in-tile, (ti, st, u)] flat NT*T cols
                    pt_sb = ptp.tile([128, NT * T], bf16, tag="pt", name="pt")
                    pt_v = pt_sb[:].rearrange("p (ti st u) -> p ti st u",
                                              ti=NT, st=ST, u=128)
                    for ti in range(NT):
                        p_f32 = psp.tile([128, T], f32, tag="p32", name="p32")
                        dacc = [dsp.tile([128, 1], f32, tag=f"da{hw}", name=f"da{hw}")
                                for hw in range(2)]
                        for hw in range(2):
                            lg = lgp.tile([128, 1024], f32, tag="lg", name="lg")
                            for half in range(2):
                                cols = slice(hw * 1024 + half * 512,
                                             hw * 1024 + (half + 1) * 512)
                                for hc in range(2):
                                    nc.tensor.matmul(
                                        lg[:, half * 512:(half + 1) * 512],
                                        qT[(qh, hc)][:, ti * 128:(ti + 1) * 128],
                                        kt_sb[(kh, hc)][:, cols],
                                        start=(hc == 0), stop=(hc == 1))
                            nc.vector.tensor_tensor(
                                lg[:], lg[:], apen_sb[ti][:, hw * 1024:(hw + 1) * 1024],
                                add_op)
                            nc.scalar.activation(
                                p_f32[:, hw * 1024:(hw + 1) * 1024], lg[:], Exp,
                                accum_out=dacc[hw][:])
                        den = dsp.tile([128, 1], f32, tag="den", name="den")
                        nc.vector.tensor_tensor(den[:], dacc[0][:], dacc[1][:], add_op)
                        rcp = dsp.tile([128, 1], f32, tag="rcp", name="rcp")
                        nc.vector.reciprocal(rcp[:], den[:])
                        p_bf = psp.tile([128, T], bf16, tag="pbf", name="pbf")
                        nc.vector.tensor_scalar_mul(p_bf[:], p_f32[:], rcp[:])
                        for g in range(4):
                            trt = trp.tile([128, 512], bf16, tag="tr", name="trt")
                            for u4 in range(4):
                                st = g * 4 + u4
                                nc.tensor.transpose(
                                    trt[:, u4 * 128:(u4 + 1) * 128],
                                    p_bf[:, st * 128:(st + 1) * 128], ident[:])
                            dst = pt_sb[:, ti * T + g * 512: ti * T + (g + 1) * 512]
                            if g % 2 == 0:
                                nc.scalar.copy(dst, trt[:])
                            else:
                                nc.vector.tensor_copy(dst, trt[:])
                    enc_ps = [encp.tile([128, CHUNK], f32, tag="enc", name="encps")
                              for _ in range(2)]
                    for st in range(ST):
                        for half in range(2):
                            nc.tensor.matmul(
                                enc_ps[half][:],
                                vt_sb[st][:, kh * H + half * 128:kh * H + (half + 1) * 128],
                                pt_v[:, :, st, :],
                                start=(st == 0), stop=(st == ST - 1))
                    for half in range(2):
                        et = encsbp.tile([128, CHUNK], bf16, tag=f"enc{qh}_{half}",
                                         name=f"enc{qh}_{half}")
                        nc.scalar.copy(et[:], enc_ps[half][:])
                        encT[(qh, half)] = et

        # ================= output projection =================
        with tc.tile_pool(name="wo_p", bufs=1) as wop, \
             tc.tile_pool(name="o_ps", bufs=2, space="PSUM") as pop, \
             tc.tile_pool(name="o_sb", bufs=3) as osbp:
            wo_sb = []
            for c16 in range(16):
                wt = wop.tile([128, D], bf16, tag=f"wo{c16}", name=f"wo{c16}")
                nc.sync.dma_start(wt[:], wo_e[c16 * 128:(c16 + 1) * 128, :])
                wo_sb.append(wt)
            for dc5 in range(5):
                for ti in range(NT):
                    po = pop.tile([128, 512], f32, tag="po", name="po")
                    for c16 in range(16):
                        nc.tensor.matmul(
                            po[:], encT[(c16 // 2, c16 % 2)][:, ti * 128:(ti + 1) * 128],
                            wo_sb[c16][:, dc5 * 512:(dc5 + 1) * 512],
                            start=(c16 == 0), stop=(c16 == 15))
                    ob = osbp.tile([128, 512], f32, tag="ob", name="ob")
                    nc.scalar.copy(ob[:], po[:])
                    nc.sync.dma_start(
                        out_e[ti * 128:(ti + 1) * 128, dc5 * 512:(dc5 + 1) * 512], ob[:])

    nc.compile()
    return nc


def _get_nc():
    if "nc" not in _BUILD_CACHE:
        _BUILD_CACHE["nc"] = _build()
    return _BUILD_CACHE["nc"]


# ------------------------------------------------------------------- driver
def kernel(x, positions, attn_mask, w_q, w_kv, q_scale, k_scale, w_o):
    x = np.asarray(x, dtype=np.float32)
    positions = np.asarray(positions)
    w_q = np.asarray(w_q, dtype=np.float32)
    w_kv = np.asarray(w_kv, dtype=np.float32)
    q_scale = np.asarray(q_scale, dtype=np.float32)
    k_scale = np.asarray(k_scale, dtype=np.float32)
    w_o = np.asarray(w_o, dtype=np.float32)

    in_maps = _prep_in_maps(x, positions, w_q, w_kv, q_scale, k_scale, w_o)
    nc = _get_nc()
    from concourse import bass2jax
    results = bass2jax.run_bass_via_pjrt(nc, in_maps, n_cores=NCORES)
    out = np.empty((B, T, D), np.float32)
    for c in range(NCORES):
        b, j = c // 4, c % 4
        out[b, CHUNK * j:CHUNK * (j + 1), :] = results[c]["out"]
    return out


# revision 12
# speedup vs baseline: 1.0703x; 1.0703x over previous
"""Trainium2 Bass kernel for sliding-window GQA attention (nn_Attention_9861244911852).

Sharding: 8 cores = 2 batches x 4 sequence chunks of 512 rows.
Each core: q/k/v projections (own 512 rows, all heads, bf16 matmuls, f32 psum),
fused rmsnorm+rope via host tables, kv AllGather across the 4-core batch group,
full-T masked attention (host-computed additive masks keep the graph core-uniform),
output projection. Host concatenates the disjoint [512, 2560] output chunks.
"""
import sys

sys.path.insert(0, '/opt/trn_rl_repo')

import numpy as np
import ml_dtypes

B, T, D, N, KH, H = 2, 2048, 2560, 8, 4, 256
WINDOW = 1024
ROPE_BASE = 10000.0
CHUNK = 512            # query rows per core
NCORES = 8
GROUPS = [[0, 1, 2, 3], [4, 5, 6, 7]]
NEG = np.float32(-1.0e38)
NT = CHUNK // 128      # 4 q-tiles per core
ST = T // 128          # 16 s-tiles (full batch)
DC = D // 128          # 20 contraction chunks
DP = H // 2            # 128 rope pairs
BF = ml_dtypes.bfloat16

_BUILD_CACHE = {}


# ----------------------------------------------------------------- host prep
def _perm():
    return np.concatenate([np.arange(0, H, 2), np.arange(1, H, 2)])


def _prep_shared(w_q, w_kv, q_scale, k_scale, w_o):
    p = _perm()
    wq = np.ascontiguousarray(w_q[:, :, p].transpose(1, 0, 2).reshape(D, N * H).astype(BF))
    wk = w_kv[0][:, :, p].transpose(1, 0, 2).reshape(D, KH * H).astype(BF)
    wv = w_kv[1].transpose(1, 0, 2).reshape(D, KH * H).astype(BF)
    wkv = np.ascontiguousarray(np.concatenate([wk, wv], axis=1))   # [D, 2048]
    wo = np.ascontiguousarray(w_o.reshape(N * H, D).astype(BF))
    qs = q_scale[p].astype(np.float32)
    ks = k_scale[p].astype(np.float32)
    return wq, wkv, wo, qs, ks


def _rope_tabs(pos, qs, ks):
    """pos [CHUNK] int32 -> [8, 128, CHUNK] f32 tables (qA qB qC qD kA kB kC kD).

    Permuted-head layout: row i of the top half holds component 2i, bottom 2i+1.
    q tables fold in q_scale and H**-0.5; k tables fold k_scale.
    """
    inv = (1.0 / ROPE_BASE ** (np.arange(DP, dtype=np.float32) / np.float32(DP))).astype(np.float32)
    ang = pos.astype(np.float32)[None, :] * inv[:, None]          # [128, CHUNK]
    c = np.cos(ang).astype(np.float32)
    s = np.sin(ang).astype(np.float32)
    s16 = np.float32(H ** -0.5)
    qt, qb = qs[:DP, None], qs[DP:, None]
    kt, kb = ks[:DP, None], ks[DP:, None]
    return np.stack([c * qt * s16, -s * qb * s16, s * qt * s16, c * qb * s16,
                     c * kt, -s * kb, s * kt, c * kb]).astype(np.float32)


def _apen(j):
    """Additive attention mask for seq-chunk j: [NT, 128, T] f32 (0 or NEG)."""
    t = (CHUNK * j + np.arange(CHUNK)).reshape(NT, 128)
    s = np.arange(T)
    d = t[:, :, None] - s[None, None, :]
    valid = (d >= 0) & (d < WINDOW)
    return np.where(valid, np.float32(0.0), NEG).astype(np.float32)


def _prep_in_maps(x, positions, w_q, w_kv, q_scale, k_scale, w_o):
    wq, wkv, wo, qs, ks = _prep_shared(w_q, w_kv, q_scale, k_scale, w_o)
    ident = np.eye(128, dtype=np.float32).astype(BF)
    in_maps = []
    for c in range(NCORES):
        b, j = c // 4, c % 4
        rows = slice(CHUNK * j, CHUNK * (j + 1))
        xt = np.ascontiguousarray(x[b, rows, :].T.astype(BF))
        rope = _rope_tabs(np.asarray(positions)[b, rows], qs, ks)
        in_maps.append({
            "xt": xt, "wq": wq, "wkv": wkv, "wo": wo,
            "rope": rope, "apen": _apen(j),
            "ident": ident,
            "onesc": np.ones((128, 1), np.float32),
            "onesr": np.ones((1, 128), np.float32),
        })
    return in_maps


# --------------------------------------------------------------- bass kernel
def _build():
    import concourse.bass as bass
    import concourse.mybir as mybir
    from concourse import bacc, tile

    f32 = mybir.dt.float32
    bf16 = mybir.dt.bfloat16

    nc = bacc.Bacc(None, target_bir_lowering=False)

    xt_e = nc.declare_dram_parameter("xt", [D, CHUNK], bf16, isOutput=False)
    wq_e = nc.declare_dram_parameter("wq", [D, N * H], bf16, isOutput=False)
    wkv_e = nc.declare_dram_parameter("wkv", [D, 2 * KH * H], bf16, isOutput=False)
    wo_e = nc.declare_dram_parameter("wo", [N * H, D], bf16, isOutput=False)
    rope_e = nc.declare_dram_parameter("rope", [8, 128, CHUNK], f32, isOutput=False)
    apen_e = nc.declare_dram_parameter("apen", [NT, 128, T], f32, isOutput=False)
    ident_e = nc.declare_dram_parameter("ident", [128, 128], bf16, isOutput=False)
    onesc_e = nc.declare_dram_parameter("onesc", [128, 1], f32, isOutput=False)
    onesr_e = nc.declare_dram_parameter("onesr", [1, 128], f32, isOutput=False)
    out_e = nc.declare_dram_parameter("out", [CHUNK, D], f32, isOutput=True)

    k_local = nc.dram_tensor("k_local", [KH * H, CHUNK], bf16)   # [1024 h, 512 s]
    v_local = nc.dram_tensor("v_local", [CHUNK, KH * H], bf16)   # [512 s, 1024 h]
    k_ag = nc.dram_tensor("k_ag", [4 * KH * H, CHUNK], bf16)     # rank-major
    v_ag = nc.dram_tensor("v_ag", [4 * CHUNK, KH * H], bf16)

    Exp = mybir.ActivationFunctionType.Exp
    Sqrt = mybir.ActivationFunctionType.Sqrt
    mult = mybir.AluOpType.mult
    add_op = mybir.AluOpType.add

    with tile.TileContext(nc) as tc:
      with tc.tile_pool(name="const", bufs=1) as constp, \
           tc.tile_pool(name="qt", bufs=1) as qtp, \
           tc.tile_pool(name="enc_sb", bufs=1) as encsbp:

        onesc = constp.tile([128, 1], f32, tag="onesc")
        nc.sync.dma_start(onesc[:], onesc_e[:])
        eps_t = constp.tile([1, 1], f32, tag="eps")
        nc.gpsimd.memset(eps_t[:], 1e-6)
        onesr = constp.tile([1, 128], f32, tag="onesr")
        nc.sync.dma_start(onesr[:], onesr_e[:])
        ident = constp.tile([128, 128], bf16, tag="ident")
        nc.sync.dma_start(ident[:], ident_e[:])

        qT = {}    # (qh, hc) -> bf16 [128, CHUNK]
        encT = {}  # (qh, half) -> bf16 [128, CHUNK]

        # ================= projections + kv AllGathers =================
        with tc.tile_pool(name="xtp", bufs=1) as xtp, \
             tc.tile_pool(name="wslab", bufs=1) as wsp, \
             tc.tile_pool(name="ropep", bufs=1) as ropep, \
             tc.tile_pool(name="qkv_ps", bufs=4, space="PSUM") as qkvp, \
             tc.tile_pool(name="ss_ps", bufs=2, space="PSUM") as ssp, \
             tc.tile_pool(name="aux_ps", bufs=2, space="PSUM") as auxp, \
             tc.tile_pool(name="scr", bufs=2) as scrp, \
             tc.tile_pool(name="kvl", bufs=1) as kvlp:

            rope_sb = []
            for ri in range(8):
                rt = ropep.tile([128, CHUNK], f32, tag=f"rope{ri}", name=f"rope{ri}")
                nc.sync.dma_start(rt[:], rope_e[ri])
                rope_sb.append(rt)

            xt_sb = []
            for dc in range(DC):
                xtile = xtp.tile([128, CHUNK], bf16, tag=f"xt{dc}", name=f"xt{dc}")
                nc.sync.dma_start(xtile[:], xt_e[dc * 128:(dc + 1) * 128, :])
                xt_sb.append(xtile)

            # weight slabs: [128, 2048] rows (4KB descriptors), resident per phase
            wkv_sb = []
            for dc in range(DC):
                nb = 2 if dc < 6 else 1
                wt = wsp.tile([128, 2048], bf16, tag=f"wkv{dc}", name=f"wkv{dc}", bufs=nb)
                nc.sync.dma_start(wt[:], wkv_e[dc * 128:(dc + 1) * 128, :])
                wkv_sb.append(wt)

            def proj_unit(slabs, col0):
                """psum [128, CHUNK] = w[:, col0:col0+128].T @ xT (20 accum matmuls)."""
                ps = qkvp.tile([128, CHUNK], f32, tag="qkv", name="qkvps")
                for dc in range(DC):
                    nc.tensor.matmul(ps[:], slabs[dc][:, col0:col0 + 128], xt_sb[dc][:],
                                     start=(dc == 0), stop=(dc == DC - 1))
                return ps

            def norm_rope(p_top, p_bot, tabs, out_top, out_bot):
                """rmsnorm (f32) + rope tables + cast bf16."""
                sq_t = scrp.tile([128, CHUNK], f32, tag="sq", name="sqt")
                nc.scalar.square(sq_t[:], p_top[:])
                ss = ssp.tile([1, CHUNK], f32, tag="ss", name="ss")
                nc.tensor.matmul(ss[:], onesc[:], sq_t[:], start=True, stop=False)
                sq_b = scrp.tile([128, CHUNK], f32, tag="sq", name="sqb")
                nc.scalar.square(sq_b[:], p_bot[:])
                nc.tensor.matmul(ss[:], onesc[:], sq_b[:], start=False, stop=True)
                std = scrp.tile([1, CHUNK], f32, tag="std", name="std")
                nc.scalar.activation(std[:], ss[:], Sqrt, bias=eps_t[:], scale=1.0 / H)
                rsb = scrp.tile([1, CHUNK], f32, tag="rsb", name="rsb")
                nc.vector.reciprocal(rsb[:], std[:])
                rb = auxp.tile([128, CHUNK], f32, tag="aux", name="rb")
                nc.tensor.matmul(rb[:], onesr[:], rsb[:], start=True, stop=True)
                A, Bt, C, Dt = tabs
                t1 = scrp.tile([128, CHUNK], f32, tag="t1", name="t1")
                t2 = scrp.tile([128, CHUNK], f32, tag="t2", name="t2")
                nc.vector.tensor_tensor(t1[:], p_top[:], A[:], mult)
                nc.vector.tensor_tensor(t2[:], p_bot[:], Bt[:], mult)
                nc.vector.tensor_tensor(t1[:], t1[:], t2[:], add_op)
                nc.vector.tensor_tensor(out_top[:], t1[:], rb[:], mult)
                t3 = scrp.tile([128, CHUNK], f32, tag="t1", name="t3")
                t4 = scrp.tile([128, CHUNK], f32, tag="t2", name="t4")
                nc.vector.tensor_tensor(t3[:], p_top[:], C[:], mult)
                nc.vector.tensor_tensor(t4[:], p_bot[:], Dt[:], mult)
                nc.vector.tensor_tensor(t3[:], t3[:], t4[:], add_op)
                nc.vector.tensor_tensor(out_bot[:], t3[:], rb[:], mult)

            # ---- k projection + epilogue -> k_local -> AllGather(k)
            for kh in range(KH):
                p_top = proj_unit(wkv_sb, kh * H)
                p_bot = proj_unit(wkv_sb, kh * H + 128)
                k_top = kvlp.tile([128, CHUNK], bf16, tag="ktop")
                k_bot = kvlp.tile([128, CHUNK], bf16, tag="kbot")
                norm_rope(p_top, p_bot, rope_sb[4:8], k_top, k_bot)
                nc.sync.dma_start(k_local[kh * H:kh * H + 128, :], k_top[:])
                nc.sync.dma_start(k_local[kh * H + 128:kh * H + 256, :], k_bot[:])

            nc.gpsimd.collective_compute(
                "AllGather", mybir.AluOpType.bypass, replica_groups=GROUPS,
                ins=[k_local[:]], outs=[k_ag[:]])

            # ---- v projection -> transpose -> v_local -> AllGather(v)
            for kh in range(KH):
                for hc in range(2):
                    ps = proj_unit(wkv_sb, 1024 + kh * H + hc * 128)
                    v_sb = scrp.tile([128, CHUNK], bf16, tag="vsb", name="vsb")
                    nc.scalar.copy(v_sb[:], ps[:])
                    vt_ps = auxp.tile([128, CHUNK], bf16, tag="aux", name="vtps")
                    for stl in range(4):
                        nc.tensor.transpose(vt_ps[:, stl * 128:(stl + 1) * 128],
                                            v_sb[:, stl * 128:(stl + 1) * 128], ident[:])
                    vt_loc = scrp.tile([128, CHUNK], bf16, tag="vtsb", name="vtloc")
                    nc.scalar.copy(vt_loc[:], vt_ps[:])
                    for stl in range(4):
                        nc.sync.dma_start(
                            v_local[stl * 128:(stl + 1) * 128,
                                    kh * H + hc * 128:kh * H + (hc + 1) * 128],
                            vt_loc[:, stl * 128:(stl + 1) * 128])

            nc.gpsimd.collective_compute(
                "AllGather", mybir.AluOpType.bypass, replica_groups=GROUPS,
                ins=[v_local[:]], outs=[v_ag[:]])

            # ---- q projection (overlaps the AllGathers)
            wq_sb = []
            for dc in range(DC):
                wt = wsp.tile([128, 2048], bf16, tag=f"wkv{dc}", name=f"wq{dc}", bufs=(2 if dc < 6 else 1))
                nc.sync.dma_start(wt[:], wq_e[dc * 128:(dc + 1) * 128, :])
                wq_sb.append(wt)
            for qh in range(N):
                p_top = proj_unit(wq_sb, qh * H)
                p_bot = proj_unit(wq_sb, qh * H + 128)
                q_top = qtp.tile([128, CHUNK], bf16, tag=f"q{qh}t", name=f"q{qh}t")
                q_bot = qtp.tile([128, CHUNK], bf16, tag=f"q{qh}b", name=f"q{qh}b")
                norm_rope(p_top, p_bot, rope_sb[0:4], q_top, q_bot)
                qT[(qh, 0)] = q_top
                qT[(qh, 1)] = q_bot

        # ================= attention =================
        with tc.tile_pool(name="apen_p", bufs=1) as apenp, \
             tc.tile_pool(name="kvw", bufs=1) as kvwp, \
             tc.tile_pool(name="psoft", bufs=2) as psp, \
             tc.tile_pool(name="ptb", bufs=2) as ptp, \
             tc.tile_pool(name="lg_ps", bufs=2, space="PSUM") as lgp, \
             tc.tile_pool(name="tr_ps", bufs=2, space="PSUM") as trp, \
             tc.tile_pool(name="enc_ps", bufs=2, space="PSUM") as encp, \
             tc.tile_pool(name="dstat", bufs=4) as dsp:

            apen_sb = []
            for ti in range(NT):
                at = apenp.tile([128, T], f32, tag=f"ap{ti}", name=f"ap{ti}")
                nc.sync.dma_start(at[:], apen_e[ti])
                apen_sb.append(at)

            # kT: 8 tiles [128, T] over (kh, hc); vT: 16 tiles [128, 1024]
            kt_sb = {}
            for kh in range(KH):
                for hc in range(2):
                    kt = kvwp.tile([128, T], bf16, tag=f"kt{kh}_{hc}", name=f"kt{kh}_{hc}")
                    for r in range(4):
                        nc.sync.dma_start(
                            kt[:, r * CHUNK:(r + 1) * CHUNK],
                            k_ag[r * 1024 + kh * H + hc * 128:
                                 r * 1024 + kh * H + (hc + 1) * 128, :])
                    kt_sb[(kh, hc)] = kt
            vt_sb = []
            for st in range(ST):
                r, stl = st // 4, st % 4
                vt = kvwp.tile([128, KH * H], bf16, tag=f"vt{st}", name=f"vt{st}")
                nc.sync.dma_start(
                    vt[:], v_ag[r * CHUNK + stl * 128:r * CHUNK + (stl + 1) * 128, :])
                vt_sb.append(vt)

            for kh in range(KH):
                for qh in (2 * kh, 2 * kh + 1):
                    # pt layout: [128 s-in-tile, (ti, st, u)] flat NT*T cols
                    pt_sb = ptp.tile([128, NT * T], bf16, tag="pt", name="pt")
                    pt_v = pt_sb[:].rearrange("p (ti st u) -> p ti st u",
                                              ti=NT, st=ST, u=128)
                    for ti in range(NT):
                        p_f32 = psp.tile([128, T], f32, tag="p32", name="p32")
                        dacc = [dsp.tile([128, 1], f32, tag=f"da{hw}", name=f"da{hw}")
                                for hw in range(2)]
                        for hw in range(2):
                            lg = lgp.tile([128, 1024], f32, tag="lg", name="lg")
                            for half in range(2):
                                cols = slice(hw * 1024 + half * 512,
                                             hw * 1024 + (half + 1) * 512)
                                for hc in range(2):
                                    nc.tensor.matmul(
                                        lg[:, half * 512:(half + 1) * 512],
                                        qT[(qh, hc)][:, ti * 128:(ti + 1) * 128],
                                        kt_sb[(kh, hc)][:, cols],
                                        start=(hc == 0), stop=(hc == 1))
                            nc.vector.tensor_tensor(
                                lg[:], lg[:], apen_sb[ti][:, hw * 1024:(hw + 1) * 1024],
                                add_op)
                            nc.scalar.activation(
                                p_f32[:, hw * 1024:(hw + 1) * 1024], lg[:], Exp,
                                accum_out=dacc[hw][:])
                        den = dsp.tile([128, 1], f32, tag="den", name="den")
                        nc.vector.tensor_tensor(den[:], dacc[0][:], dacc[1][:], add_op)
                        rcp = dsp.tile([128, 1], f32, tag="rcp", name="rcp")
                        nc.vector.reciprocal(rcp[:], den[:])
                        p_bf = psp.tile([128, T], bf16, tag="pbf", name="pbf")
                        nc.vector.tensor_scalar_mul(p_bf[:], p_f32[:], rcp[:])
                        for g in range(4):
                            trt = trp.tile([128, 512], bf16, tag="tr", name="trt")
                            for u4 in range(4):
                                st = g * 4 + u4
                                nc.tensor.transpose(
                                    trt[:, u4 * 128:(u4 + 1) * 128],
                                    p_bf[:, st * 128:(st + 1) * 128], ident[:])
                            dst = pt_sb[:, ti * T + g * 512: ti * T + (g + 1) * 512]
                            if g % 2 == 0:
                                nc.scalar.copy(dst, trt[:])
                            else:
                                nc.vector.tensor_copy(dst, trt[:])
                    enc_ps = [encp.tile([128, CHUNK], f32, tag="enc", name="encps")
                              for _ in range(2)]
                    for st in range(ST):
                        for half in range(2):
                            nc.tensor.matmul(
                                enc_ps[half][:],
                                vt_sb[st][:, kh * H + half * 128:kh * H + (half + 1) * 128],
                                pt_v[:, :, st, :],
                                start=(st == 0), stop=(st == ST - 1))
                    for half in range(2):
                        et = encsbp.tile([128, CHUNK], bf16, tag=f"enc{qh}_{half}",
                                         name=f"enc{qh}_{half}")
                        nc.scalar.copy(et[:], enc_ps[half][:])
                        encT[(qh, half)] = et

        # ================= output projection =================
        with tc.tile_pool(name="wo_p", bufs=1) as wop, \
             tc.tile_pool(name="o_ps", bufs=2, space="PSUM") as pop, \
             tc.tile_pool(name="o_sb", bufs=3) as osbp:
            wo_sb = []
            for c16 in range(16):
                wt = wop.tile([128, D], bf16, tag=f"wo{c16}", name=f"wo{c16}")
                nc.sync.dma_start(wt[:], wo_e[c16 * 128:(c16 + 1) * 128, :])
                wo_sb.append(wt)
            for dc5 in range(5):
                for ti in range(NT):
                    po = pop.tile([128, 512], f32, tag="po", name="po")
                    for c16 in range(16):
                        nc.tensor.matmul(
                            po[:], encT[(c16 // 2, c16 % 2)][:, ti * 128:(ti + 1) * 128],
                            wo_sb[c16][:, dc5 * 512:(dc5 + 1) * 512],
                            start=(c16 == 0), stop=(c16 == 15))
                    ob = osbp.tile([128, 512], f32, tag="ob", name="ob")
                    nc.scalar.copy(ob[:], po[:])
                    nc.sync.dma_start(
                        out_e[ti * 128:(ti + 1) * 128, dc5 * 512:(dc5 + 1) * 512], ob[:])

    nc.compile()
    return nc


def _get_nc():
    if "nc" not in _BUILD_CACHE:
        _BUILD_CACHE["nc"] = _build()
    return _BUILD_CACHE["nc"]


# ------------------------------------------------------------------- driver
def kernel(x, positions, attn_mask, w_q, w_kv, q_scale, k_scale, w_o):
    x = np.asarray(x, dtype=np.float32)
    positions = np.asarray(positions)
    w_q = np.asarray(w_q, dtype=np.float32)
    w_kv = np.asarray(w_kv, dtype=np.float32)
    q_scale = np.asarray(q_scale, dtype=np.float32)
    k_scale = np.asarray(k_scale, dtype=np.float32)
    w_o = np.asarray(w_o, dtype=np.float32)

    in_maps = _prep_in_maps(x, positions, w_q, w_kv, q_scale, k_scale, w_o)
    nc = _get_nc()
    from concourse import bass2jax
    results = bass2jax.run_bass_via_pjrt(nc, in_maps, n_cores=NCORES)
    out = np.empty((B, T, D), np.float32)
    for c in range(NCORES):
        b, j = c // 4, c % 4
        out[b, CHUNK * j:CHUNK * (j + 1), :] = results[c]["out"]
    return out
